# revision 16
# baseline (speedup 1.0000x reference)
"""Trainium2 Bass kernel for causal MultiHeadAttention (B=2, S=2048, E=1024, H=16).

Sharding: 8 cores = 2 (batch) x 4 (head groups of 4, Megatron-style).
Each core computes, for its batch b and head group g:
  - Q/K projections into transposed layout qhT/khT [256, S]  (256 = 4 heads x 64)
  - V projection into natural layout vh [S, 256] with a ones-column per head
  - causal attention with scores kept transposed [k, q]; softmax denominators
    come out of the PV matmul via the ones-column; no max-subtraction needed
    (|scores/sqrt(D)| <~ 6 so exp is well within fp32 range; masked entries are
    zeroed AFTER exp, which matches the reference's -1e9 masking exactly)
  - partial output projection attn_concat @ Wo[rows of g]  -> [S, E]
Host sums the 4 partials per batch and adds bo + bv @ Wo (the V bias commutes
through the softmax-normalized average, so it is folded on the host).

All matmul operands are float16 (full PE rate, fp32 PSUM accumulation).
The two per-head-pair score matmuls run concurrently on disjoint PE row
groups (implicit tile_position (0,0)/(64,0) from their base partitions).
Causal masking multiplies only the 128-wide diagonal wedge of each diagonal
block. Softmax normalization: denominator rides the PV ones-column; ACT
computes rs = 32*exp(-ln(den)) and the PE broadcasts it into the spare
partitions 64:128 of the attention PSUM bank, so one DVE
scalar_tensor_tensor does (attn * 1/32) * rs -> normalized fp16 atT.
Projection/wo matmuls interleave into the attention stream as PE filler.
"""

import numpy as np

B, S, E, H = 2, 2048, 1024, 16
D = E // H            # 64 head dim
HL = 4                # heads per core
CW = HL * D           # 256 local channels
P = 128
NQ = 512              # q-chunk (one fp32 PSUM bank)
KT = E // P           # 8 contraction tiles for the input projections
D1 = D + 1            # head slot in vh (+ ones column)
RSC = 32.0            # reciprocal pre-scale: rs = RSC/den, undone in the STT

_CACHE = {}


def _pin_act_table(mybir, bacc):
    """Force all activations onto one LUT set containing exp+ln+identity, so
    the ACT engine never reloads tables mid-kernel (1.3us per reload)."""
    from concourse.hw_specs import get_activation_tables

    need = {
        mybir.ActivationFunctionType.Exp,
        mybir.ActivationFunctionType.Ln,
        mybir.ActivationFunctionType.Identity,
    }
    orig = get_activation_tables("gen3")
    target = next(n for n, fs in orig.items() if need <= fs)
    pinned = {n: (fs if n == target else set()) for n, fs in orig.items()}
    bacc.get_activation_tables = lambda arch: pinned


def _build(nc_s=S, num_devices=8):
    import concourse.mybir as mybir
    import concourse.tile as tile
    from concourse import bacc

    _pin_act_table(mybir, bacc)

    f32 = mybir.dt.float32
    h16 = mybir.dt.float16
    Ln = mybir.ActivationFunctionType.Ln
    Exp = mybir.ActivationFunctionType.Exp
    MUL = mybir.AluOpType.mult

    QC = nc_s // NQ        # q-chunks
    SB = nc_s // P         # S blocks of 128

    nc = bacc.Bacc(
        "TRN2", target_bir_lowering=False, debug=False, num_devices=num_devices
    )

    def din(name, shape, dt=f32):
        return nc.dram_tensor(name, list(shape), dt, kind="ExternalInput").ap()

    xqt = din("xqt", (E, nc_s), h16)
    xkt = din("xkt", (E, nc_s), h16)
    xvt = din("xvt", (E, nc_s), h16)
    wq = din("wq", (E, CW), h16)
    wk = din("wk", (E, CW), h16)
    wv = din("wv", (E, CW), h16)
    wo = din("wo", (CW, E), h16)
    bq = din("bq", (CW,))
    bk = din("bk", (CW,))
    masks = din("masks", (P, 2 * P), h16)
    onesd = din("ones", (P, P), h16)
    out = nc.dram_tensor("out", [nc_s, E], h16, kind="ExternalOutput").ap()

    with tile.TileContext(nc) as tc:
        with (
            tc.tile_pool(name="singles", bufs=1) as singles,
            tc.tile_pool(name="xpool", bufs=6) as xpool,
            tc.tile_pool(name="exp", bufs=10) as exp_pool,
            tc.tile_pool(name="outp", bufs=4) as out_pool,
            tc.tile_pool(name="small", bufs=4) as small_pool,
            tc.tile_pool(name="proj_ps", bufs=2, space="PSUM") as proj_ps,
            tc.tile_pool(name="scores_ps", bufs=2, space="PSUM") as scores_ps,
            tc.tile_pool(name="attn_ps", bufs=2, space="PSUM") as attn_ps,
        ):
            sy = nc.sync

            # --- persistent SBUF tensors -------------------------------------
            wq_sb = singles.tile([P, KT, CW], h16, tag="wq")
            wk_sb = singles.tile([P, KT, CW], h16, tag="wk")
            wv_sb = singles.tile([P, KT, CW], h16, tag="wv")
            wo_sb = singles.tile([P, CW // P, E], h16, tag="wo")
            masks_sb = singles.tile([P, 2, P], h16, tag="masks")
            bq_sb = singles.tile([P, 2], f32, tag="bq")
            bk_sb = singles.tile([P, 2], f32, tag="bk")
            ones_sb = singles.tile([P, SB * HL], h16, tag="ones_sb")

            qhT = [singles.tile([P, nc_s], h16, name=f"qhT{m}", tag=f"qhT{m}") for m in range(2)]
            khT = [singles.tile([P, nc_s], h16, name=f"khT{m}", tag=f"khT{m}") for m in range(2)]
            atT = [singles.tile([P, nc_s], h16, name=f"atT{m}", tag=f"atT{m}") for m in range(2)]
            vh = singles.tile([P, SB, HL, D1], h16, tag="vh")

            def t_wk():
                rw = wk.rearrange("(kt p) m -> p kt m", p=P)
                sy.dma_start(out=wk_sb[:, :1, :], in_=rw[:, :1, :])
                sy.dma_start(out=wk_sb[:, 1 : KT // 2, :], in_=rw[:, 1 : KT // 2, :])
                sy.dma_start(out=wk_sb[:, KT // 2 :, :], in_=rw[:, KT // 2 :, :])
                sy.dma_start(out=bk_sb, in_=bk.rearrange("(m p) -> p m", p=P))

            def t_wv():
                sy.dma_start(out=wv_sb, in_=wv.rearrange("(kt p) m -> p kt m", p=P))

            def t_wq():
                sy.dma_start(out=wq_sb, in_=wq.rearrange("(kt p) m -> p kt m", p=P))
                sy.dma_start(out=bq_sb, in_=bq.rearrange("(m p) -> p m", p=P))

            def t_attn_consts():
                sy.dma_start(
                    out=masks_sb, in_=masks.rearrange("p (j n) -> p j n", n=P)
                )
                sy.dma_start(out=ones_sb, in_=onesd[:, 0 : SB * HL])

            def t_vh_ones():
                nc.vector.tensor_copy(
                    out=vh[:, :, :, D:D1],
                    in_=ones_sb.rearrange("p (a b) -> p a b", b=HL).unsqueeze(3),
                )

            def t_wo():
                sy.dma_start(out=wo_sb, in_=wo.rearrange("(kt p) n -> p kt n", p=P))

            # --- stage helpers (thunk-list builders) -------------------------
            def load_x_thunk(src, c, holder, key):
                def t():
                    tl = xpool.tile([P, KT, NQ], h16, name="xchunk", tag="xchunk")
                    rsrc = src.rearrange("(kt p) s -> p kt s", p=P)[
                        :, :, c * NQ : (c + 1) * NQ
                    ]
                    h = KT // 2
                    if c == 0:
                        sy.dma_start(out=tl[:, :1, :], in_=rsrc[:, :1, :])
                        sy.dma_start(out=tl[:, 1:h, :], in_=rsrc[:, 1:h, :])
                    else:
                        sy.dma_start(out=tl[:, :h, :], in_=rsrc[:, :h, :])
                    sy.dma_start(out=tl[:, h:, :], in_=rsrc[:, h:, :])
                    holder[key] = tl
                return [t]

            def proj_qk_thunks(c, holder, key, w_sb, b_sb, dstT):
                thunks = []
                pss = {}
                for m in range(2):
                    def mk_mm(m, kt):
                        def t():
                            if kt == 0:
                                pss[m] = proj_ps.tile([P, NQ], f32, name="proj", tag="proj")
                            nc.tensor.matmul(
                                pss[m],
                                w_sb[:, kt, m * P : (m + 1) * P],
                                holder[key][:, kt, :],
                                start=(kt == 0),
                                stop=(kt == KT - 1),
                            )
                        return t
                    for kt in range(KT):
                        thunks.append(mk_mm(m, kt))
                    def mk_copy(m):
                        def t():
                            nc.vector.tensor_scalar_add(
                                out=dstT[m][:, c * NQ : (c + 1) * NQ],
                                in0=pss[m],
                                scalar1=b_sb[:, m : m + 1],
                            )
                        return t
                    thunks.append(mk_copy(m))
                return thunks

            def proj_v_thunks(c, holder, key):
                """One self-contained thunk per 128-row block of vh (8 MMs +
                evacuation copy), so each can be zipped right before the
                diagonal attention step that consumes it."""
                thunks = []
                for mb in range(4):
                    j = 4 * c + mb
                    def mk_vmb(mb, j):
                        def t():
                            ps = proj_ps.tile([P, NQ], f32, name="proj", tag="proj")
                            for kt in range(KT):
                                nc.tensor.matmul(
                                    ps[:, :CW],
                                    holder[key][:, kt, mb * P : (mb + 1) * P],
                                    wv_sb[:, kt, :],
                                    start=(kt == 0),
                                    stop=(kt == KT - 1),
                                )
                            nc.vector.tensor_copy(
                                out=vh[:, j, :, 0:D],
                                in_=ps[:, :CW].rearrange("p (h d) -> p h d", h=HL),
                            )
                        return t
                    thunks.append(mk_vmb(mb, j))
                return thunks

            def attn_thunks(c, vmb_thunks=None):
                """Attention for q-chunk c, software-pipelined: scores(j+1)
                issues before PV(j), so the exp(j) latency hides behind
                scores(j+1)+PV(j-1) with no PE queue stall. vmb_thunks (hp0
                only) are zipped in just before the diagonal score whose PV
                consumes that vh block."""
                thunks = []
                nblk = 4 * (c + 1)
                scale = float(1.0 / np.sqrt(D))
                for hp in range(2):
                    ats = {}
                    exs = {}
                    def mk_sc(hp, j, exs):
                        def t():
                            jj = j - 4 * c
                            q0 = jj * P if jj > 0 else 0
                            sc2 = scores_ps.tile([P, 2, NQ], f32, name="sc2", tag="sc2")
                            for hh in range(2):
                                po = hh * D
                                nc.tensor.matmul(
                                    sc2[:, hh, q0:],
                                    khT[hp][po : po + D, j * P : (j + 1) * P],
                                    qhT[hp][po : po + D, c * NQ + q0 : (c + 1) * NQ],
                                    start=True,
                                    stop=True,
                                )
                            ex2 = exp_pool.tile([P, 2, NQ], h16, name="ex2", tag="ex2")
                            nc.scalar.activation(
                                out=ex2[:, :, q0:], in_=sc2[:, :, q0:], func=Exp,
                                scale=scale,
                            )
                            if jj >= 0:
                                # only the 128-wide diagonal wedge needs masking
                                nc.vector.tensor_mul(
                                    ex2[:, :, q0 : q0 + P],
                                    ex2[:, :, q0 : q0 + P],
                                    masks_sb,
                                )
                            exs[j] = ex2
                        return t
                    def mk_pv(hp, j, ats, exs):
                        def t():
                            if j == 0:
                                ats[0] = attn_ps.tile([D1, NQ], f32, name="attn", tag="attn")
                                ats[1] = attn_ps.tile([D1, NQ], f32, name="attn", tag="attn")
                            jj = j - 4 * c
                            q0 = jj * P if jj > 0 else 0
                            ex2 = exs.pop(j)
                            for hh in range(2):
                                nc.tensor.matmul(
                                    ats[hh][:, q0:],
                                    vh[:, j, 2 * hp + hh, :],
                                    ex2[:, hh, q0:],
                                    start=(j == 0),
                                    stop=(j == nblk - 1),
                                )
                        return t
                    for j in range(nblk):
                        if hp == 0 and vmb_thunks is not None and j >= 4 * c:
                            thunks.append(vmb_thunks[j - 4 * c])
                        thunks.append(mk_sc(hp, j, exs))
                        if j >= 1:
                            thunks.append(mk_pv(hp, j - 1, ats, exs))
                    thunks.append(mk_pv(hp, nblk - 1, ats, exs))

                    dsbs = {}
                    def mk_den(hh, ats, dsbs):
                        def t():
                            # denominator row to SBUF so gpsimd can read it
                            dsb = small_pool.tile([1, NQ], f32, name="dsb", tag="dsb")
                            nc.vector.tensor_copy(dsb, ats[hh][D : D + 1, :])
                            dsbs[hh] = dsb
                        return t
                    thunks.append(mk_den(0, ats, dsbs))
                    thunks.append(mk_den(1, ats, dsbs))

                    def mk_norm(hp, hh, ats, dsbs):
                        def t():
                            po = hh * D
                            db = small_pool.tile([D, NQ], f32, name="db", tag="db")
                            nc.gpsimd.partition_broadcast(db, dsbs[hh])
                            # 1/den on the DVE (18-bit approx; den is in
                            # [e^-6, 1e4], far from the op's edge cases) —
                            # keeps the ACT engine free for the exp stream
                            rb = small_pool.tile([D, NQ], f32, name="rb", tag="rb")
                            nc.vector.reciprocal_approx_fast(out=rb, in_=db)
                            nc.vector.tensor_mul(
                                atT[hp][po : po + D, c * NQ : (c + 1) * NQ],
                                ats[hh][0:D, :],
                                rb,
                            )
                        return t
                    thunks.append(mk_norm(hp, 0, ats, dsbs))
                    thunks.append(mk_norm(hp, 1, ats, dsbs))
                return thunks

            def wo_thunks(c):
                thunks = []
                for mb in range(4):
                    ms = 4 * c + mb
                    for n in range(2):
                        def mk(ms, n):
                            def t():
                                ps = proj_ps.tile([P, NQ], f32, name="proj", tag="proj")
                                for kt in range(CW // P):
                                    nc.tensor.matmul(
                                        ps,
                                        atT[kt][:, ms * P : (ms + 1) * P],
                                        wo_sb[:, kt, n * NQ : (n + 1) * NQ],
                                        start=(kt == 0),
                                        stop=(kt == CW // P - 1),
                                    )
                                ot = out_pool.tile([P, NQ], h16, name="ot", tag="ot")
                                nc.vector.tensor_copy(ot, ps)
                                sy.dma_start(
                                    out=out[
                                        ms * P : (ms + 1) * P, n * NQ : (n + 1) * NQ
                                    ],
                                    in_=ot,
                                )
                            return t
                        thunks.append(mk(ms, n))
                return thunks

            def wo_tail_thunks(c):
                """Final-chunk wo. All 8 units' kt=0 matmuls (pair-0 atT,
                ready since mid-round) issue first, filling the PE while
                pair-1's normalization chain runs; then all kt=1 matmuls.
                PSUM: units 0-1 proj pool, 2-5 scores banks, 6-7 the attn
                banks as they free mid-chain. Evacuation alternates DVE/ACT."""
                thunks = []
                units = [(4 * c + mb, n) for mb in range(4) for n in range(2)]
                pss = {}
                def mk_kt0(i, ms, n, pss):
                    def t():
                        if i < 2:
                            pss[i] = proj_ps.tile(
                                [P, NQ], f32, name="proj", tag="proj"
                            )
                        elif i < 6:
                            if i % 2 == 0:
                                pss["sc"] = scores_ps.tile(
                                    [P, 2, NQ], f32, name="sc2", tag="sc2"
                                )
                            pss[i] = pss["sc"][:, i % 2, :]
                        else:
                            pss[i] = attn_ps.tile(
                                [P, NQ], f32, name="attn", tag="attn"
                            )
                        nc.tensor.matmul(
                            pss[i],
                            atT[0][:, ms * P : (ms + 1) * P],
                            wo_sb[:, 0, n * NQ : (n + 1) * NQ],
                            start=True,
                            stop=False,
                        )
                    return t
                def mk_kt1(i, ms, n, pss):
                    def t():
                        nc.tensor.matmul(
                            pss[i],
                            atT[1][:, ms * P : (ms + 1) * P],
                            wo_sb[:, 1, n * NQ : (n + 1) * NQ],
                            start=False,
                            stop=True,
                        )
                        ot = out_pool.tile([P, NQ], h16, name="ot", tag="ot")
                        if i % 2 == 0:
                            nc.vector.tensor_copy(ot, pss[i])
                        else:
                            nc.scalar.copy(ot, pss[i])
                        sy.dma_start(
                            out=out[ms * P : (ms + 1) * P, n * NQ : (n + 1) * NQ],
                            in_=ot,
                        )
                    return t
                for i, (ms, n) in enumerate(units):
                    thunks.append(mk_kt0(i, ms, n, pss))
                for i, (ms, n) in enumerate(units):
                    thunks.append(mk_kt1(i, ms, n, pss))
                return thunks

            def emit_interleaved(primary, filler):
                fi = 0
                n = max(len(primary), 1)
                f = len(filler)
                for i, t in enumerate(primary):
                    t()
                    while fi * n < f * (i + 1):
                        filler[fi]()
                        fi += 1
                for t in filler[fi:]:
                    t()

            def t_warmup():
                # ~5us of throwaway matmuls while the first DMAs stream in:
                # carries the PE through the HAM SHORT window so the real
                # prologue projections run at full clock
                wsb = singles.tile([P, P], h16, tag="warm")
                nc.vector.memset(wsb, 0.03125)
                wps = proj_ps.tile([P, NQ], f32, name="proj", tag="proj")
                for _ in range(44):
                    nc.tensor.matmul(wps[:, 0:P], wsb, wsb, start=True, stop=True)

            # --- main schedule ----------------------------------------------
            # All prologue DMAs issue first, in need-order, so the DMA queues
            # stream ahead of the PE while the warm-up matmuls run.
            holder = {}
            dma_thunks = (
                [t_wk]
                + load_x_thunk(xkt, 0, holder, ("xk", 0))
                + [t_wv]
                + load_x_thunk(xvt, 0, holder, ("xv", 0))
                + [t_wq]
                + load_x_thunk(xqt, 0, holder, ("xq", 0))
                + [t_attn_consts, t_wo]
            )
            compute_thunks = (
                proj_qk_thunks(0, holder, ("xk", 0), wk_sb, bk_sb, khT)
                + [t_vh_ones]
                + proj_qk_thunks(0, holder, ("xq", 0), wq_sb, bq_sb, qhT)
            )
            for t in dma_thunks:
                t()
            t_warmup()
            for t in compute_thunks:
                t()
            kv_deferred = {}
            for c in range(QC):
                kv_filler = kv_deferred.pop(c, [])
                filler = []
                if c == 2:
                    filler += wo_thunks(0)
                if c == 3:
                    filler += wo_thunks(1) + wo_thunks(2)
                if c + 1 < QC:
                    filler += load_x_thunk(xkt, c + 1, holder, ("xk", c + 1))
                    filler += load_x_thunk(xvt, c + 1, holder, ("xv", c + 1))
                    filler += load_x_thunk(xqt, c + 1, holder, ("xq", c + 1))
                    filler += proj_qk_thunks(
                        c + 1, holder, ("xq", c + 1), wq_sb, bq_sb, qhT
                    )
                    # K projection of chunk c+1 runs inside round c+1 itself
                    # (khT isn't needed until its diagonal), keeping PE filler
                    # in the late, exp-heavy rounds
                    kv_deferred[c + 1] = proj_qk_thunks(
                        c + 1, holder, ("xk", c + 1), wk_sb, bk_sb, khT
                    )
                # V projection of chunk c zips into the diagonal section
                vmb = proj_v_thunks(c, holder, ("xv", c))
                att = attn_thunks(c, vmb_thunks=vmb)
                # seg1 ends where hp0's diagonal section begins (the first
                # zipped v thunk); kv_filler must complete within seg1
                nsc_off = 2 * (4 * c) - (1 if c > 0 else 0)
                seg1, seg2 = att[:nsc_off] if c > 0 else [], att[nsc_off:] if c > 0 else att
                emit_interleaved(seg1, kv_filler)
                # hold back a quarter of the filler to keep PE fed through the
                # end-of-round normalization chains
                cut = (3 * len(filler)) // 4
                emit_interleaved(seg2[:-8], filler[:cut])
                emit_interleaved(seg2[-8:], filler[cut:])
            for t in wo_tail_thunks(QC - 1):
                t()

    nc.compile()
    return nc


def _get_nc(nc_s=S):
    if nc_s not in _CACHE:
        _CACHE[nc_s] = _build(nc_s)
    return _CACHE[nc_s]


def make_masks():
    # one 128x128 lower-triangle wedge (same for every diagonal block),
    # duplicated for the two heads an exp tile carries
    kl = np.arange(P)[:, None]
    t = np.arange(P)[None, :]
    m = (t >= kl).astype(np.float32)
    return np.concatenate([m, m], axis=1)


def make_in_maps(q, k, v, Wq, bq, Wk, bk, Wv, Wo):
    masks = make_masks()
    in_maps = []
    for core in range(8):
        b, g = divmod(core, 4)
        cs = slice(g * CW, (g + 1) * CW)
        in_maps.append(
            {
                "xqt": np.ascontiguousarray(q[b].T).astype(np.float16),
                "xkt": np.ascontiguousarray(k[b].T).astype(np.float16),
                "xvt": np.ascontiguousarray(v[b].T).astype(np.float16),
                "wq": np.ascontiguousarray(Wq[:, cs]).astype(np.float16),
                "wk": np.ascontiguousarray(Wk[:, cs]).astype(np.float16),
                "wv": np.ascontiguousarray(Wv[:, cs]).astype(np.float16),
                "wo": np.ascontiguousarray(Wo[cs, :]).astype(np.float16),
                "bq": np.ascontiguousarray(bq[cs]),
                "bk": np.ascontiguousarray(bk[cs]),
                "masks": masks.astype(np.float16),
                "ones": np.ones((P, P), np.float16),
            }
        )
    return in_maps


def run(q, k, v, Wq, bq, Wk, bk, Wv, bv, Wo, bo, **run_kwargs):
    """Returns (output, BassKernelResults)."""
    from concourse.bass_utils import run_bass_kernel_spmd

    q, k, v = (np.asarray(x, np.float32) for x in (q, k, v))
    Wo = np.asarray(Wo, np.float32)
    bv = np.asarray(bv, np.float32)
    nc = _get_nc()
    in_maps = make_in_maps(
        q, k, v,
        np.asarray(Wq, np.float32), np.asarray(bq, np.float32),
        np.asarray(Wk, np.float32), np.asarray(bk, np.float32),
        np.asarray(Wv, np.float32),
        Wo,
    )
    res = run_bass_kernel_spmd(nc, in_maps, list(range(8)), **run_kwargs)
    out = np.zeros((B, S, E), np.float32)
    for core in range(8):
        out[core // 4] += res.results[core]["out"].astype(np.float32)
    # V bias commutes through the softmax average; Wo is linear in it.
    out += (np.asarray(bo, np.float32) + bv @ Wo)[None, None, :]
    return out, res


def kernel(q, k, v, Wq, bq, Wk, bk, Wv, bv, Wo, bo):
    return run(q, k, v, Wq, bq, Wk, bk, Wv, bv, Wo, bo)[0]


# revision 19
# speedup vs baseline: 1.0037x; 1.0037x over previous
"""Trainium2 Bass kernel for causal MultiHeadAttention (B=2, S=2048, E=1024, H=16).

Sharding: 8 cores = 2 (batch) x 4 (head groups of 4, Megatron-style).
Each core computes, for its batch b and head group g:
  - Q/K projections into transposed layout qhT/khT [256, S]  (256 = 4 heads x 64)
  - V projection into natural layout vh [S, 256] with a ones-column per head
  - causal attention with scores kept transposed [k, q]; softmax denominators
    come out of the PV matmul via the ones-column; no max-subtraction needed
    (|scores/sqrt(D)| <~ 6 so exp is well within fp32 range; masked entries are
    zeroed AFTER exp, which matches the reference's -1e9 masking exactly)
  - partial output projection attn_concat @ Wo[rows of g]  -> [S, E]
Host sums the 4 partials per batch and adds bo + bv @ Wo (the V bias commutes
through the softmax-normalized average, so it is folded on the host).

All matmul operands are float16 (full PE rate, fp32 PSUM accumulation).
The two per-head-pair score matmuls run concurrently on disjoint PE row
groups (implicit tile_position (0,0)/(64,0) from their base partitions).
Causal masking multiplies only the 128-wide diagonal wedge of each diagonal
block. Softmax normalization: denominator rides the PV ones-column; ACT
computes rs = 32*exp(-ln(den)) and the PE broadcasts it into the spare
partitions 64:128 of the attention PSUM bank, so one DVE
scalar_tensor_tensor does (attn * 1/32) * rs -> normalized fp16 atT.
Projection/wo matmuls interleave into the attention stream as PE filler.
"""

import numpy as np

B, S, E, H = 2, 2048, 1024, 16
D = E // H            # 64 head dim
HL = 4                # heads per core
CW = HL * D           # 256 local channels
P = 128
NQ = 512              # q-chunk (one fp32 PSUM bank)
KT = E // P           # 8 contraction tiles for the input projections
D1 = D + 1            # head slot in vh (+ ones column)
RSC = 32.0            # reciprocal pre-scale: rs = RSC/den, undone in the STT

_CACHE = {}


def _pin_act_table(mybir, bacc):
    """Force all activations onto one LUT set containing exp+ln+identity, so
    the ACT engine never reloads tables mid-kernel (1.3us per reload)."""
    from concourse.hw_specs import get_activation_tables

    need = {
        mybir.ActivationFunctionType.Exp,
        mybir.ActivationFunctionType.Ln,
        mybir.ActivationFunctionType.Identity,
    }
    orig = get_activation_tables("gen3")
    target = next(n for n, fs in orig.items() if need <= fs)
    pinned = {n: (fs if n == target else set()) for n, fs in orig.items()}
    bacc.get_activation_tables = lambda arch: pinned


def _build(nc_s=S, num_devices=8):
    import concourse.mybir as mybir
    import concourse.tile as tile
    from concourse import bacc

    _pin_act_table(mybir, bacc)

    f32 = mybir.dt.float32
    h16 = mybir.dt.float16
    Ln = mybir.ActivationFunctionType.Ln
    Exp = mybir.ActivationFunctionType.Exp
    MUL = mybir.AluOpType.mult

    QC = nc_s // NQ        # q-chunks
    SB = nc_s // P         # S blocks of 128

    nc = bacc.Bacc(
        "TRN2", target_bir_lowering=False, debug=False, num_devices=num_devices
    )

    def din(name, shape, dt=f32):
        return nc.dram_tensor(name, list(shape), dt, kind="ExternalInput").ap()

    xqt = din("xqt", (E, nc_s), h16)
    xkt = din("xkt", (E, nc_s), h16)
    xvt = din("xvt", (E, nc_s), h16)
    wq = din("wq", (E, CW), h16)
    wk = din("wk", (E, CW), h16)
    wv = din("wv", (E, CW), h16)
    wo = din("wo", (CW, E), h16)
    bq = din("bq", (CW,))
    bk = din("bk", (CW,))
    masks = din("masks", (P, 2 * P), h16)
    onesd = din("ones", (P, P), h16)
    out = nc.dram_tensor("out", [nc_s, E], h16, kind="ExternalOutput").ap()

    with tile.TileContext(nc) as tc:
        with (
            tc.tile_pool(name="singles", bufs=1) as singles,
            tc.tile_pool(name="xpool", bufs=6) as xpool,
            tc.tile_pool(name="exp", bufs=10) as exp_pool,
            tc.tile_pool(name="outp", bufs=4) as out_pool,
            tc.tile_pool(name="small", bufs=4) as small_pool,
            tc.tile_pool(name="proj_ps", bufs=2, space="PSUM") as proj_ps,
            tc.tile_pool(name="scores_ps", bufs=2, space="PSUM") as scores_ps,
            tc.tile_pool(name="attn_ps", bufs=2, space="PSUM") as attn_ps,
        ):
            sy = nc.sync

            # --- persistent SBUF tensors -------------------------------------
            wq_sb = singles.tile([P, KT, CW], h16, tag="wq")
            wk_sb = singles.tile([P, KT, CW], h16, tag="wk")
            wv_sb = singles.tile([P, KT, CW], h16, tag="wv")
            wo_sb = singles.tile([P, CW // P, E], h16, tag="wo")
            masks_sb = singles.tile([P, 2, P], h16, tag="masks")
            bq_sb = singles.tile([P, 2], f32, tag="bq")
            bk_sb = singles.tile([P, 2], f32, tag="bk")
            ones_sb = singles.tile([P, SB * HL], h16, tag="ones_sb")

            qhT = [singles.tile([P, nc_s], h16, name=f"qhT{m}", tag=f"qhT{m}") for m in range(2)]
            khT = [singles.tile([P, nc_s], h16, name=f"khT{m}", tag=f"khT{m}") for m in range(2)]
            atT = [singles.tile([P, nc_s], h16, name=f"atT{m}", tag=f"atT{m}") for m in range(2)]
            vh = singles.tile([P, SB, HL, D1], h16, tag="vh")

            def t_wk():
                rw = wk.rearrange("(kt p) m -> p kt m", p=P)
                sy.dma_start(out=wk_sb[:, :1, :], in_=rw[:, :1, :])
                sy.dma_start(out=wk_sb[:, 1 : KT // 2, :], in_=rw[:, 1 : KT // 2, :])
                sy.dma_start(out=wk_sb[:, KT // 2 :, :], in_=rw[:, KT // 2 :, :])
                sy.dma_start(out=bk_sb, in_=bk.rearrange("(m p) -> p m", p=P))

            def t_wv():
                sy.dma_start(out=wv_sb, in_=wv.rearrange("(kt p) m -> p kt m", p=P))

            def t_wq():
                sy.dma_start(out=wq_sb, in_=wq.rearrange("(kt p) m -> p kt m", p=P))
                sy.dma_start(out=bq_sb, in_=bq.rearrange("(m p) -> p m", p=P))

            def t_attn_consts():
                sy.dma_start(
                    out=masks_sb, in_=masks.rearrange("p (j n) -> p j n", n=P)
                )
                sy.dma_start(out=ones_sb, in_=onesd[:, 0 : SB * HL])

            def t_vh_ones():
                nc.vector.tensor_copy(
                    out=vh[:, :, :, D:D1],
                    in_=ones_sb.rearrange("p (a b) -> p a b", b=HL).unsqueeze(3),
                )

            def t_wo():
                sy.dma_start(out=wo_sb, in_=wo.rearrange("(kt p) n -> p kt n", p=P))

            # --- stage helpers (thunk-list builders) -------------------------
            def load_x_thunk(src, c, holder, key):
                def t():
                    tl = xpool.tile([P, KT, NQ], h16, name="xchunk", tag="xchunk")
                    rsrc = src.rearrange("(kt p) s -> p kt s", p=P)[
                        :, :, c * NQ : (c + 1) * NQ
                    ]
                    h = KT // 2
                    if c == 0:
                        sy.dma_start(out=tl[:, :1, :], in_=rsrc[:, :1, :])
                        sy.dma_start(out=tl[:, 1:h, :], in_=rsrc[:, 1:h, :])
                    else:
                        sy.dma_start(out=tl[:, :h, :], in_=rsrc[:, :h, :])
                    sy.dma_start(out=tl[:, h:, :], in_=rsrc[:, h:, :])
                    holder[key] = tl
                return [t]

            def proj_qk_thunks(c, holder, key, w_sb, b_sb, dstT):
                thunks = []
                pss = {}
                for m in range(2):
                    def mk_mm(m, kt):
                        def t():
                            if kt == 0:
                                pss[m] = proj_ps.tile([P, NQ], f32, name="proj", tag="proj")
                            nc.tensor.matmul(
                                pss[m],
                                w_sb[:, kt, m * P : (m + 1) * P],
                                holder[key][:, kt, :],
                                start=(kt == 0),
                                stop=(kt == KT - 1),
                            )
                        return t
                    for kt in range(KT):
                        thunks.append(mk_mm(m, kt))
                    def mk_copy(m):
                        def t():
                            nc.vector.tensor_scalar_add(
                                out=dstT[m][:, c * NQ : (c + 1) * NQ],
                                in0=pss[m],
                                scalar1=b_sb[:, m : m + 1],
                            )
                        return t
                    thunks.append(mk_copy(m))
                return thunks

            def proj_v_thunks(c, holder, key):
                """One self-contained thunk per 128-row block of vh (8 MMs +
                evacuation copy), so each can be zipped right before the
                diagonal attention step that consumes it."""
                thunks = []
                for mb in range(4):
                    j = 4 * c + mb
                    def mk_vmb(mb, j):
                        def t():
                            ps = proj_ps.tile([P, NQ], f32, name="proj", tag="proj")
                            for kt in range(KT):
                                nc.tensor.matmul(
                                    ps[:, :CW],
                                    holder[key][:, kt, mb * P : (mb + 1) * P],
                                    wv_sb[:, kt, :],
                                    start=(kt == 0),
                                    stop=(kt == KT - 1),
                                )
                            nc.vector.tensor_copy(
                                out=vh[:, j, :, 0:D],
                                in_=ps[:, :CW].rearrange("p (h d) -> p h d", h=HL),
                            )
                        return t
                    thunks.append(mk_vmb(mb, j))
                return thunks

            def attn_thunks(c, vmb_thunks=None):
                """Attention for q-chunk c, software-pipelined: scores(j+1)
                issues before PV(j), so the exp(j) latency hides behind
                scores(j+1)+PV(j-1) with no PE queue stall. vmb_thunks (hp0
                only) are zipped in just before the diagonal score whose PV
                consumes that vh block."""
                thunks = []
                nblk = 4 * (c + 1)
                scale = float(1.0 / np.sqrt(D))
                for hp in range(2):
                    ats = {}
                    exs = {}
                    def mk_sc(hp, j, exs):
                        def t():
                            jj = j - 4 * c
                            q0 = jj * P if jj > 0 else 0
                            sc2 = scores_ps.tile([P, 2, NQ], f32, name="sc2", tag="sc2")
                            for hh in range(2):
                                po = hh * D
                                nc.tensor.matmul(
                                    sc2[:, hh, q0:],
                                    khT[hp][po : po + D, j * P : (j + 1) * P],
                                    qhT[hp][po : po + D, c * NQ + q0 : (c + 1) * NQ],
                                    start=True,
                                    stop=True,
                                )
                            ex2 = exp_pool.tile([P, 2, NQ], h16, name="ex2", tag="ex2")
                            nc.scalar.activation(
                                out=ex2[:, :, q0:], in_=sc2[:, :, q0:], func=Exp,
                                scale=scale,
                            )
                            if jj >= 0:
                                # only the 128-wide diagonal wedge needs masking
                                nc.vector.tensor_mul(
                                    ex2[:, :, q0 : q0 + P],
                                    ex2[:, :, q0 : q0 + P],
                                    masks_sb,
                                )
                            exs[j] = ex2
                        return t
                    def mk_pv(hp, j, ats, exs):
                        def t():
                            if j == 0:
                                ats[0] = attn_ps.tile([D1, NQ], f32, name="attn", tag="attn")
                                ats[1] = attn_ps.tile([D1, NQ], f32, name="attn", tag="attn")
                            jj = j - 4 * c
                            q0 = jj * P if jj > 0 else 0
                            ex2 = exs.pop(j)
                            for hh in range(2):
                                nc.tensor.matmul(
                                    ats[hh][:, q0:],
                                    vh[:, j, 2 * hp + hh, :],
                                    ex2[:, hh, q0:],
                                    start=(j == 0),
                                    stop=(j == nblk - 1),
                                )
                        return t
                    for j in range(nblk):
                        if hp == 0 and vmb_thunks is not None and j >= 4 * c:
                            thunks.append(vmb_thunks[j - 4 * c])
                        thunks.append(mk_sc(hp, j, exs))
                        if j >= 1:
                            thunks.append(mk_pv(hp, j - 1, ats, exs))
                    thunks.append(mk_pv(hp, nblk - 1, ats, exs))

                    dsbs = {}
                    def mk_den(hh, ats, dsbs):
                        def t():
                            # denominator row to SBUF so gpsimd can read it;
                            # hh=0 on ACT, hh=1 on DVE so the two norm chains
                            # start in parallel
                            dsb = small_pool.tile([1, NQ], f32, name="dsb", tag="dsb")
                            if hh == 0:
                                nc.scalar.copy(dsb, ats[hh][D : D + 1, :])
                            else:
                                nc.vector.tensor_copy(dsb, ats[hh][D : D + 1, :])
                            dsbs[hh] = dsb
                        return t
                    thunks.append(mk_den(0, ats, dsbs))
                    thunks.append(mk_den(1, ats, dsbs))

                    def mk_norm(hp, hh, ats, dsbs):
                        def t():
                            po = hh * D
                            db = small_pool.tile([D, NQ], f32, name="db", tag="db")
                            nc.gpsimd.partition_broadcast(db, dsbs[hh])
                            # 1/den on the DVE (18-bit approx; den is in
                            # [e^-6, 1e4], far from the op's edge cases) —
                            # keeps the ACT engine free for the exp stream
                            rb = small_pool.tile([D, NQ], f32, name="rb", tag="rb")
                            nc.vector.reciprocal_approx_fast(out=rb, in_=db)
                            nc.vector.tensor_mul(
                                atT[hp][po : po + D, c * NQ : (c + 1) * NQ],
                                ats[hh][0:D, :],
                                rb,
                            )
                        return t
                    thunks.append(mk_norm(hp, 0, ats, dsbs))
                    thunks.append(mk_norm(hp, 1, ats, dsbs))
                return thunks

            def wo_thunks(c):
                thunks = []
                for mb in range(4):
                    ms = 4 * c + mb
                    for n in range(2):
                        def mk(ms, n):
                            def t():
                                ps = proj_ps.tile([P, NQ], f32, name="proj", tag="proj")
                                for kt in range(CW // P):
                                    nc.tensor.matmul(
                                        ps,
                                        atT[kt][:, ms * P : (ms + 1) * P],
                                        wo_sb[:, kt, n * NQ : (n + 1) * NQ],
                                        start=(kt == 0),
                                        stop=(kt == CW // P - 1),
                                    )
                                ot = out_pool.tile([P, NQ], h16, name="ot", tag="ot")
                                nc.vector.tensor_copy(ot, ps)
                                sy.dma_start(
                                    out=out[
                                        ms * P : (ms + 1) * P, n * NQ : (n + 1) * NQ
                                    ],
                                    in_=ot,
                                )
                            return t
                        thunks.append(mk(ms, n))
                return thunks

            def wo_tail_thunks(c):
                """Final-chunk wo. All 8 units' kt=0 matmuls (pair-0 atT,
                ready since mid-round) issue first, filling the PE while
                pair-1's normalization chain runs; then all kt=1 matmuls.
                PSUM: units 0-1 proj pool, 2-5 scores banks, 6-7 the attn
                banks as they free mid-chain. Evacuation alternates DVE/ACT."""
                thunks = []
                units = [(4 * c + mb, n) for mb in range(4) for n in range(2)]
                pss = {}
                def mk_kt0(i, ms, n, pss):
                    def t():
                        if i < 2:
                            pss[i] = proj_ps.tile(
                                [P, NQ], f32, name="proj", tag="proj"
                            )
                        elif i < 6:
                            if i % 2 == 0:
                                pss["sc"] = scores_ps.tile(
                                    [P, 2, NQ], f32, name="sc2", tag="sc2"
                                )
                            pss[i] = pss["sc"][:, i % 2, :]
                        else:
                            pss[i] = attn_ps.tile(
                                [P, NQ], f32, name="attn", tag="attn"
                            )
                        nc.tensor.matmul(
                            pss[i],
                            atT[0][:, ms * P : (ms + 1) * P],
                            wo_sb[:, 0, n * NQ : (n + 1) * NQ],
                            start=True,
                            stop=False,
                        )
                    return t
                def mk_kt1(i, ms, n, pss):
                    def t():
                        nc.tensor.matmul(
                            pss[i],
                            atT[1][:, ms * P : (ms + 1) * P],
                            wo_sb[:, 1, n * NQ : (n + 1) * NQ],
                            start=False,
                            stop=True,
                        )
                        ot = out_pool.tile([P, NQ], h16, name="ot", tag="ot")
                        if i % 2 == 0:
                            nc.vector.tensor_copy(ot, pss[i])
                        else:
                            nc.scalar.copy(ot, pss[i])
                        sy.dma_start(
                            out=out[ms * P : (ms + 1) * P, n * NQ : (n + 1) * NQ],
                            in_=ot,
                        )
                    return t
                for i, (ms, n) in enumerate(units):
                    thunks.append(mk_kt0(i, ms, n, pss))
                for i, (ms, n) in enumerate(units):
                    thunks.append(mk_kt1(i, ms, n, pss))
                return thunks

            def emit_interleaved(primary, filler):
                fi = 0
                n = max(len(primary), 1)
                f = len(filler)
                for i, t in enumerate(primary):
                    t()
                    while fi * n < f * (i + 1):
                        filler[fi]()
                        fi += 1
                for t in filler[fi:]:
                    t()

            def t_warmup():
                # ~5us of throwaway matmuls while the first DMAs stream in:
                # carries the PE through the HAM SHORT window so the real
                # prologue projections run at full clock
                wsb = singles.tile([P, P], h16, tag="warm")
                nc.vector.memset(wsb, 0.03125)
                wps = proj_ps.tile([P, NQ], f32, name="proj", tag="proj")
                for _ in range(52):
                    nc.tensor.matmul(wps[:, 0:P], wsb, wsb, start=True, stop=True)

            # --- main schedule ----------------------------------------------
            # All prologue DMAs issue first, in need-order, so the DMA queues
            # stream ahead of the PE while the warm-up matmuls run.
            holder = {}
            dma_thunks = (
                [t_wk]
                + load_x_thunk(xkt, 0, holder, ("xk", 0))
                + [t_attn_consts, t_wv]
                + load_x_thunk(xvt, 0, holder, ("xv", 0))
                + [t_wq]
                + load_x_thunk(xqt, 0, holder, ("xq", 0))
                + [t_wo]
            )
            compute_thunks = (
                proj_qk_thunks(0, holder, ("xk", 0), wk_sb, bk_sb, khT)
                + [t_vh_ones]
                + proj_qk_thunks(0, holder, ("xq", 0), wq_sb, bq_sb, qhT)
            )
            for t in dma_thunks:
                t()
            t_warmup()
            for t in compute_thunks:
                t()
            kv_deferred = {}
            for c in range(QC):
                kv_filler = kv_deferred.pop(c, [])
                filler = []
                if c == 2:
                    filler += wo_thunks(0)
                if c == 3:
                    filler += wo_thunks(1) + wo_thunks(2)
                if c + 1 < QC:
                    filler += load_x_thunk(xkt, c + 1, holder, ("xk", c + 1))
                    filler += load_x_thunk(xvt, c + 1, holder, ("xv", c + 1))
                    filler += load_x_thunk(xqt, c + 1, holder, ("xq", c + 1))
                    filler += proj_qk_thunks(
                        c + 1, holder, ("xq", c + 1), wq_sb, bq_sb, qhT
                    )
                    # K projection of chunk c+1 runs inside round c+1 itself
                    # (khT isn't needed until its diagonal), keeping PE filler
                    # in the late, exp-heavy rounds
                    kv_deferred[c + 1] = proj_qk_thunks(
                        c + 1, holder, ("xk", c + 1), wk_sb, bk_sb, khT
                    )
                # V projection of chunk c zips into the diagonal section
                vmb = proj_v_thunks(c, holder, ("xv", c))
                att = attn_thunks(c, vmb_thunks=vmb)
                # seg1 ends where hp0's diagonal section begins (the first
                # zipped v thunk); kv_filler must complete within seg1
                nsc_off = 2 * (4 * c) - (1 if c > 0 else 0)
                seg1, seg2 = att[:nsc_off] if c > 0 else [], att[nsc_off:] if c > 0 else att
                emit_interleaved(seg1, kv_filler)
                # hold back a quarter of the filler to keep PE fed through the
                # end-of-round normalization chains
                cut = (3 * len(filler)) // 4
                emit_interleaved(seg2[:-8], filler[:cut])
                emit_interleaved(seg2[-8:], filler[cut:])
            for t in wo_tail_thunks(QC - 1):
                t()

    nc.compile()
    return nc


def _get_nc(nc_s=S):
    if nc_s not in _CACHE:
        _CACHE[nc_s] = _build(nc_s)
    return _CACHE[nc_s]


def make_masks():
    # one 128x128 lower-triangle wedge (same for every diagonal block),
    # duplicated for the two heads an exp tile carries
    kl = np.arange(P)[:, None]
    t = np.arange(P)[None, :]
    m = (t >= kl).astype(np.float32)
    return np.concatenate([m, m], axis=1)


def make_in_maps(q, k, v, Wq, bq, Wk, bk, Wv, Wo):
    masks = make_masks()
    in_maps = []
    for core in range(8):
        b, g = divmod(core, 4)
        cs = slice(g * CW, (g + 1) * CW)
        in_maps.append(
            {
                "xqt": np.ascontiguousarray(q[b].T).astype(np.float16),
                "xkt": np.ascontiguousarray(k[b].T).astype(np.float16),
                "xvt": np.ascontiguousarray(v[b].T).astype(np.float16),
                "wq": np.ascontiguousarray(Wq[:, cs]).astype(np.float16),
                "wk": np.ascontiguousarray(Wk[:, cs]).astype(np.float16),
                "wv": np.ascontiguousarray(Wv[:, cs]).astype(np.float16),
                "wo": np.ascontiguousarray(Wo[cs, :]).astype(np.float16),
                "bq": np.ascontiguousarray(bq[cs]),
                "bk": np.ascontiguousarray(bk[cs]),
                "masks": masks.astype(np.float16),
                "ones": np.ones((P, P), np.float16),
            }
        )
    return in_maps


def run(q, k, v, Wq, bq, Wk, bk, Wv, bv, Wo, bo, **run_kwargs):
    """Returns (output, BassKernelResults)."""
    from concourse.bass_utils import run_bass_kernel_spmd

    q, k, v = (np.asarray(x, np.float32) for x in (q, k, v))
    Wo = np.asarray(Wo, np.float32)
    bv = np.asarray(bv, np.float32)
    nc = _get_nc()
    in_maps = make_in_maps(
        q, k, v,
        np.asarray(Wq, np.float32), np.asarray(bq, np.float32),
        np.asarray(Wk, np.float32), np.asarray(bk, np.float32),
        np.asarray(Wv, np.float32),
        Wo,
    )
    res = run_bass_kernel_spmd(nc, in_maps, list(range(8)), **run_kwargs)
    out = np.zeros((B, S, E), np.float32)
    for core in range(8):
        out[core // 4] += res.results[core]["out"].astype(np.float32)
    # V bias commutes through the softmax average; Wo is linear in it.
    out += (np.asarray(bo, np.float32) + bv @ Wo)[None, None, :]
    return out, res


def kernel(q, k, v, Wq, bq, Wk, bk, Wv, bv, Wo, bo):
    return run(q, k, v, Wq, bq, Wk, bk, Wv, bv, Wo, bo)[0]


# revision 23
# speedup vs baseline: 1.0145x; 1.0108x over previous
"""Trainium2 Bass kernel for causal MultiHeadAttention (B=2, S=2048, E=1024, H=16).

Sharding: 8 cores = 2 (batch) x 4 (head groups of 4, Megatron-style).
Each core computes, for its batch b and head group g:
  - Q/K projections into transposed layout qhT/khT [256, S]  (256 = 4 heads x 64)
  - V projection into natural layout vh [S, 256] with a ones-column per head
  - causal attention with scores kept transposed [k, q]; softmax denominators
    come out of the PV matmul via the ones-column; no max-subtraction needed
    (|scores/sqrt(D)| <~ 6 so exp is well within fp32 range; masked entries are
    zeroed AFTER exp, which matches the reference's -1e9 masking exactly)
  - partial output projection attn_concat @ Wo[rows of g]  -> [S, E]
Host sums the 4 partials per batch and adds bo + bv @ Wo (the V bias commutes
through the softmax-normalized average, so it is folded on the host).

All matmul operands are float16 (full PE rate, fp32 PSUM accumulation).
The two per-head-pair score matmuls run concurrently on disjoint PE row
groups (implicit tile_position (0,0)/(64,0) from their base partitions).
Causal masking multiplies only the 128-wide diagonal wedge of each diagonal
block. Softmax normalization: denominator rides the PV ones-column; ACT
computes rs = 32*exp(-ln(den)) and the PE broadcasts it into the spare
partitions 64:128 of the attention PSUM bank, so one DVE
scalar_tensor_tensor does (attn * 1/32) * rs -> normalized fp16 atT.
Projection/wo matmuls interleave into the attention stream as PE filler.
"""

import numpy as np

B, S, E, H = 2, 2048, 1024, 16
D = E // H            # 64 head dim
HL = 4                # heads per core
CW = HL * D           # 256 local channels
P = 128
NQ = 512              # q-chunk (one fp32 PSUM bank)
KT = E // P           # 8 contraction tiles for the input projections
D1 = D + 1            # head slot in vh (+ ones column)
RSC = 32.0            # reciprocal pre-scale: rs = RSC/den, undone in the STT

_CACHE = {}


def _pin_act_table(mybir, bacc):
    """Force all activations onto one LUT set containing exp+ln+identity, so
    the ACT engine never reloads tables mid-kernel (1.3us per reload)."""
    from concourse.hw_specs import get_activation_tables

    need = {
        mybir.ActivationFunctionType.Exp,
        mybir.ActivationFunctionType.Ln,
        mybir.ActivationFunctionType.Identity,
    }
    orig = get_activation_tables("gen3")
    target = next(n for n, fs in orig.items() if need <= fs)
    pinned = {n: (fs if n == target else set()) for n, fs in orig.items()}
    bacc.get_activation_tables = lambda arch: pinned


def _build(nc_s=S, num_devices=8):
    import concourse.mybir as mybir
    import concourse.tile as tile
    from concourse import bacc

    _pin_act_table(mybir, bacc)

    f32 = mybir.dt.float32
    h16 = mybir.dt.float16
    Ln = mybir.ActivationFunctionType.Ln
    Exp = mybir.ActivationFunctionType.Exp
    MUL = mybir.AluOpType.mult

    QC = nc_s // NQ        # q-chunks
    SB = nc_s // P         # S blocks of 128

    nc = bacc.Bacc(
        "TRN2", target_bir_lowering=False, debug=False, num_devices=num_devices
    )

    def din(name, shape, dt=f32):
        return nc.dram_tensor(name, list(shape), dt, kind="ExternalInput").ap()

    xqt = din("xqt", (E, nc_s), h16)
    xkt = din("xkt", (E, nc_s), h16)
    xvt = din("xvt", (E, nc_s), h16)
    wq = din("wq", (E, CW), h16)
    wk = din("wk", (E, CW), h16)
    wv = din("wv", (E, CW), h16)
    wo = din("wo", (CW, E), h16)
    bq = din("bq", (CW,))
    bk = din("bk", (CW,))
    masks = din("masks", (P, 2 * P), h16)
    onesd = din("ones", (P, P), h16)
    out = nc.dram_tensor("out", [nc_s, E], h16, kind="ExternalOutput").ap()

    with tile.TileContext(nc) as tc:
        with (
            tc.tile_pool(name="singles", bufs=1) as singles,
            tc.tile_pool(name="xpool", bufs=6) as xpool,
            tc.tile_pool(name="exp", bufs=10) as exp_pool,
            tc.tile_pool(name="outp", bufs=4) as out_pool,
            tc.tile_pool(name="small", bufs=4) as small_pool,
            tc.tile_pool(name="proj_ps", bufs=2, space="PSUM") as proj_ps,
            tc.tile_pool(name="scores_ps", bufs=2, space="PSUM") as scores_ps,
            tc.tile_pool(name="attn_ps", bufs=2, space="PSUM") as attn_ps,
        ):
            sy = nc.sync

            # --- persistent SBUF tensors -------------------------------------
            wq_sb = singles.tile([P, KT, CW], h16, tag="wq")
            wk_sb = singles.tile([P, KT, CW], h16, tag="wk")
            wv_sb = singles.tile([P, KT, CW], h16, tag="wv")
            wo_sb = singles.tile([P, CW // P, E], h16, tag="wo")
            masks_sb = singles.tile([P, 2, P], h16, tag="masks")
            bq_sb = singles.tile([P, 2], f32, tag="bq")
            bk_sb = singles.tile([P, 2], f32, tag="bk")
            ones_sb = singles.tile([P, SB * HL], h16, tag="ones_sb")

            qhT = [singles.tile([P, nc_s], h16, name=f"qhT{m}", tag=f"qhT{m}") for m in range(2)]
            khT = [singles.tile([P, nc_s], h16, name=f"khT{m}", tag=f"khT{m}") for m in range(2)]
            atT = [singles.tile([P, nc_s], h16, name=f"atT{m}", tag=f"atT{m}") for m in range(2)]
            vh = singles.tile([P, SB, HL, D1], h16, tag="vh")

            def t_wk():
                rw = wk.rearrange("(kt p) m -> p kt m", p=P)
                sy.dma_start(out=wk_sb[:, :1, :], in_=rw[:, :1, :])
                sy.dma_start(out=wk_sb[:, 1 : KT // 2, :], in_=rw[:, 1 : KT // 2, :])
                sy.dma_start(out=wk_sb[:, KT // 2 :, :], in_=rw[:, KT // 2 :, :])
                sy.dma_start(out=bk_sb, in_=bk.rearrange("(m p) -> p m", p=P))

            def t_wv():
                sy.dma_start(out=wv_sb, in_=wv.rearrange("(kt p) m -> p kt m", p=P))

            def t_wq():
                sy.dma_start(out=wq_sb, in_=wq.rearrange("(kt p) m -> p kt m", p=P))
                sy.dma_start(out=bq_sb, in_=bq.rearrange("(m p) -> p m", p=P))

            def t_attn_consts():
                sy.dma_start(
                    out=masks_sb, in_=masks.rearrange("p (j n) -> p j n", n=P)
                )
                sy.dma_start(out=ones_sb, in_=onesd[:, 0 : SB * HL])

            def t_vh_ones():
                nc.vector.tensor_copy(
                    out=vh[:, :, :, D:D1],
                    in_=ones_sb.rearrange("p (a b) -> p a b", b=HL).unsqueeze(3),
                )

            def t_wo():
                sy.dma_start(out=wo_sb, in_=wo.rearrange("(kt p) n -> p kt n", p=P))

            # --- stage helpers (thunk-list builders) -------------------------
            def load_x_thunk(src, c, holder, key):
                def t():
                    tl = xpool.tile([P, KT, NQ], h16, name="xchunk", tag="xchunk")
                    rsrc = src.rearrange("(kt p) s -> p kt s", p=P)[
                        :, :, c * NQ : (c + 1) * NQ
                    ]
                    h = KT // 2
                    if c == 0:
                        sy.dma_start(out=tl[:, :1, :], in_=rsrc[:, :1, :])
                        sy.dma_start(out=tl[:, 1:h, :], in_=rsrc[:, 1:h, :])
                    else:
                        sy.dma_start(out=tl[:, :h, :], in_=rsrc[:, :h, :])
                    sy.dma_start(out=tl[:, h:, :], in_=rsrc[:, h:, :])
                    holder[key] = tl
                return [t]

            def proj_qk_thunks(c, holder, key, w_sb, b_sb, dstT):
                thunks = []
                pss = {}
                for m in range(2):
                    def mk_mm(m, kt):
                        def t():
                            if kt == 0:
                                pss[m] = proj_ps.tile([P, NQ], f32, name="proj", tag="proj")
                            nc.tensor.matmul(
                                pss[m],
                                w_sb[:, kt, m * P : (m + 1) * P],
                                holder[key][:, kt, :],
                                start=(kt == 0),
                                stop=(kt == KT - 1),
                            )
                        return t
                    for kt in range(KT):
                        thunks.append(mk_mm(m, kt))
                    def mk_copy(m):
                        def t():
                            nc.vector.tensor_scalar_add(
                                out=dstT[m][:, c * NQ : (c + 1) * NQ],
                                in0=pss[m],
                                scalar1=b_sb[:, m : m + 1],
                            )
                        return t
                    thunks.append(mk_copy(m))
                return thunks

            def proj_v_thunks(c, holder, key):
                """One self-contained thunk per 128-row block of vh (8 MMs +
                evacuation copy), so each can be zipped right before the
                diagonal attention step that consumes it."""
                thunks = []
                for mb in range(4):
                    j = 4 * c + mb
                    def mk_vmb(mb, j):
                        def t():
                            ps = proj_ps.tile([P, NQ], f32, name="proj", tag="proj")
                            for kt in range(KT):
                                nc.tensor.matmul(
                                    ps[:, :CW],
                                    holder[key][:, kt, mb * P : (mb + 1) * P],
                                    wv_sb[:, kt, :],
                                    start=(kt == 0),
                                    stop=(kt == KT - 1),
                                )
                            nc.vector.tensor_copy(
                                out=vh[:, j, :, 0:D],
                                in_=ps[:, :CW].rearrange("p (h d) -> p h d", h=HL),
                            )
                        return t
                    thunks.append(mk_vmb(mb, j))
                return thunks

            def attn_thunks(c, vmb_thunks=None):
                """Attention for q-chunk c, software-pipelined: scores(j+1)
                issues before PV(j), so the exp(j) latency hides behind
                scores(j+1)+PV(j-1) with no PE queue stall. vmb_thunks (hp0
                only) are zipped in just before the diagonal score whose PV
                consumes that vh block."""
                thunks = []
                nblk = 4 * (c + 1)
                scale = float(1.0 / np.sqrt(D))
                for hp in range(2):
                    ats = {}
                    exs = {}
                    def mk_sc(hp, j, exs):
                        def t():
                            jj = j - 4 * c
                            q0 = jj * P if jj > 0 else 0
                            sc2 = scores_ps.tile([P, 2, NQ], f32, name="sc2", tag="sc2")
                            for hh in range(2):
                                po = hh * D
                                nc.tensor.matmul(
                                    sc2[:, hh, q0:],
                                    khT[hp][po : po + D, j * P : (j + 1) * P],
                                    qhT[hp][po : po + D, c * NQ + q0 : (c + 1) * NQ],
                                    start=True,
                                    stop=True,
                                )
                            ex2 = exp_pool.tile([P, 2, NQ], h16, name="ex2", tag="ex2")
                            nc.scalar.activation(
                                out=ex2[:, :, q0:], in_=sc2[:, :, q0:], func=Exp,
                                scale=scale,
                            )
                            if jj >= 0:
                                # only the 128-wide diagonal wedge needs masking
                                nc.vector.tensor_mul(
                                    ex2[:, :, q0 : q0 + P],
                                    ex2[:, :, q0 : q0 + P],
                                    masks_sb,
                                )
                            exs[j] = ex2
                        return t
                    def mk_pv(hp, j, ats, exs):
                        def t():
                            if j == 0:
                                ats[0] = attn_ps.tile([D1, NQ], f32, name="attn", tag="attn")
                                ats[1] = attn_ps.tile([D1, NQ], f32, name="attn", tag="attn")
                            jj = j - 4 * c
                            q0 = jj * P if jj > 0 else 0
                            ex2 = exs.pop(j)
                            for hh in range(2):
                                nc.tensor.matmul(
                                    ats[hh][:, q0:],
                                    vh[:, j, 2 * hp + hh, :],
                                    ex2[:, hh, q0:],
                                    start=(j == 0),
                                    stop=(j == nblk - 1),
                                )
                        return t
                    for j in range(nblk):
                        if hp == 0 and vmb_thunks is not None and j >= 4 * c:
                            thunks.append(vmb_thunks[j - 4 * c])
                        thunks.append(mk_sc(hp, j, exs))
                        if j >= 1:
                            thunks.append(mk_pv(hp, j - 1, ats, exs))
                    thunks.append(mk_pv(hp, nblk - 1, ats, exs))

                    rss = {}
                    atus = {}
                    def mk_rs1(ats, rss):
                        def t():
                            # hh=1 reciprocal on the DVE: copy den row out of
                            # PSUM, 18-bit approx reciprocal, cast to fp16
                            # (den is in [e^-6, 1e4], far from edge cases)
                            dn = small_pool.tile([1, NQ], f32, name="dn", tag="dn")
                            nc.vector.tensor_copy(dn, ats[1][D : D + 1, :])
                            rc = small_pool.tile([1, NQ], f32, name="rc", tag="rc")
                            nc.vector.reciprocal_approx_fast(out=rc, in_=dn)
                            rs = small_pool.tile([1, NQ], h16, name="rs", tag="rs")
                            nc.vector.tensor_copy(rs, rc)
                            rss[1] = rs
                        return t
                    def mk_rs0(ats, rss):
                        def t():
                            # hh=0 reciprocal on the ACT: exp(-ln(den))
                            ls = small_pool.tile([1, NQ], f32, name="ls", tag="ls")
                            nc.scalar.activation(
                                out=ls, in_=ats[0][D : D + 1, :], func=Ln, scale=1.0
                            )
                            rs = small_pool.tile([1, NQ], h16, name="rs0", tag="rs0")
                            nc.scalar.activation(
                                out=rs, in_=ls, func=Exp, scale=-1.0,
                            )
                            rss[0] = rs
                        return t
                    def mk_atu(hh, ats, atus):
                        def t():
                            # attn values out of PSUM (frees the attn bank;
                            # the norm multiply may read only one PSUM input)
                            atu = small_pool.tile([D, NQ], h16, name="atu", tag="atu")
                            if hh == 0:
                                nc.scalar.copy(atu, ats[hh][0:D, :])
                            else:
                                nc.vector.tensor_copy(atu, ats[hh][0:D, :])
                            atus[hh] = atu
                        return t
                    thunks.append(mk_rs1(ats, rss))
                    thunks.append(mk_rs0(ats, rss))
                    thunks.append(mk_atu(0, ats, atus))
                    thunks.append(mk_atu(1, ats, atus))

                    def mk_norm(hp, atus, rss):
                        def t():
                            # both reciprocals broadcast across partitions by
                            # two tiny K=1 PE matmuls, one per bank of a
                            # borrowed scores tile (partitions 0:64 only —
                            # higher col-groups hit a PE tiling HW bug)
                            rbt = scores_ps.tile([P, 2, NQ], f32, name="sc2", tag="sc2")
                            for hh in range(2):
                                nc.tensor.matmul(
                                    rbt[0:D, hh, :], ones_sb[0:1, 0:D], rss[hh],
                                    start=True, stop=True,
                                )
                            for hh in range(2):
                                po = hh * D
                                nc.vector.tensor_mul(
                                    atT[hp][po : po + D, c * NQ : (c + 1) * NQ],
                                    atus[hh],
                                    rbt[0:D, hh, :],
                                )
                        return t
                    thunks.append(mk_norm(hp, atus, rss))
                return thunks

            def wo_thunks(c):
                thunks = []
                for mb in range(4):
                    ms = 4 * c + mb
                    for n in range(2):
                        def mk(ms, n):
                            def t():
                                ps = proj_ps.tile([P, NQ], f32, name="proj", tag="proj")
                                for kt in range(CW // P):
                                    nc.tensor.matmul(
                                        ps,
                                        atT[kt][:, ms * P : (ms + 1) * P],
                                        wo_sb[:, kt, n * NQ : (n + 1) * NQ],
                                        start=(kt == 0),
                                        stop=(kt == CW // P - 1),
                                    )
                                ot = out_pool.tile([P, NQ], h16, name="ot", tag="ot")
                                nc.vector.tensor_copy(ot, ps)
                                sy.dma_start(
                                    out=out[
                                        ms * P : (ms + 1) * P, n * NQ : (n + 1) * NQ
                                    ],
                                    in_=ot,
                                )
                            return t
                        thunks.append(mk(ms, n))
                return thunks

            def wo_tail_thunks(c):
                """Final-chunk wo. All 8 units' kt=0 matmuls (pair-0 atT,
                ready since mid-round) issue first, filling the PE while
                pair-1's normalization chain runs; then all kt=1 matmuls.
                PSUM: units 0-1 proj pool, 2-5 scores banks, 6-7 the attn
                banks as they free mid-chain. Evacuation alternates DVE/ACT."""
                thunks = []
                units = [(4 * c + mb, n) for mb in range(4) for n in range(2)]
                pss = {}
                def mk_kt0(i, ms, n, pss):
                    def t():
                        if i < 2:
                            pss[i] = proj_ps.tile(
                                [P, NQ], f32, name="proj", tag="proj"
                            )
                        elif i < 6:
                            if i % 2 == 0:
                                pss["sc"] = scores_ps.tile(
                                    [P, 2, NQ], f32, name="sc2", tag="sc2"
                                )
                            pss[i] = pss["sc"][:, i % 2, :]
                        else:
                            pss[i] = attn_ps.tile(
                                [P, NQ], f32, name="attn", tag="attn"
                            )
                        nc.tensor.matmul(
                            pss[i],
                            atT[0][:, ms * P : (ms + 1) * P],
                            wo_sb[:, 0, n * NQ : (n + 1) * NQ],
                            start=True,
                            stop=False,
                        )
                    return t
                def mk_kt1(i, ms, n, pss):
                    def t():
                        nc.tensor.matmul(
                            pss[i],
                            atT[1][:, ms * P : (ms + 1) * P],
                            wo_sb[:, 1, n * NQ : (n + 1) * NQ],
                            start=False,
                            stop=True,
                        )
                        ot = out_pool.tile([P, NQ], h16, name="ot", tag="ot")
                        if i % 2 == 0:
                            nc.vector.tensor_copy(ot, pss[i])
                        else:
                            nc.scalar.copy(ot, pss[i])
                        sy.dma_start(
                            out=out[ms * P : (ms + 1) * P, n * NQ : (n + 1) * NQ],
                            in_=ot,
                        )
                    return t
                for i, (ms, n) in enumerate(units):
                    thunks.append(mk_kt0(i, ms, n, pss))
                for i, (ms, n) in enumerate(units):
                    thunks.append(mk_kt1(i, ms, n, pss))
                return thunks

            def emit_interleaved(primary, filler):
                fi = 0
                n = max(len(primary), 1)
                f = len(filler)
                for i, t in enumerate(primary):
                    t()
                    while fi * n < f * (i + 1):
                        filler[fi]()
                        fi += 1
                for t in filler[fi:]:
                    t()

            def t_warmup():
                # ~5us of throwaway matmuls while the first DMAs stream in:
                # carries the PE through the HAM SHORT window so the real
                # prologue projections run at full clock
                wsb = singles.tile([P, P], h16, tag="warm")
                nc.vector.memset(wsb, 0.03125)
                wps = proj_ps.tile([P, NQ], f32, name="proj", tag="proj")
                for _ in range(52):
                    nc.tensor.matmul(wps[:, 0:P], wsb, wsb, start=True, stop=True)

            # --- main schedule ----------------------------------------------
            # All prologue DMAs issue first, in need-order, so the DMA queues
            # stream ahead of the PE while the warm-up matmuls run.
            holder = {}
            dma_thunks = (
                [t_wk]
                + load_x_thunk(xkt, 0, holder, ("xk", 0))
                + [t_attn_consts, t_wv]
                + load_x_thunk(xvt, 0, holder, ("xv", 0))
                + [t_wq]
                + load_x_thunk(xqt, 0, holder, ("xq", 0))
            )
            compute_thunks = (
                proj_qk_thunks(0, holder, ("xk", 0), wk_sb, bk_sb, khT)
                + [t_vh_ones]
                + proj_qk_thunks(0, holder, ("xq", 0), wq_sb, bq_sb, qhT)
            )
            for t in dma_thunks:
                t()
            t_warmup()
            for t in compute_thunks:
                t()
            kv_deferred = {}
            for c in range(QC):
                kv_filler = kv_deferred.pop(c, [])
                filler = []
                if c == 2:
                    filler += wo_thunks(0)
                if c == 3:
                    filler += wo_thunks(1) + wo_thunks(2)
                post = []
                if c + 1 < QC:
                    # xq first: its projection is this round's PE filler;
                    # xk/xv aren't consumed until round c+1
                    filler += load_x_thunk(xqt, c + 1, holder, ("xq", c + 1))
                    filler += load_x_thunk(xkt, c + 1, holder, ("xk", c + 1))
                    filler += load_x_thunk(xvt, c + 1, holder, ("xv", c + 1))
                    qp = proj_qk_thunks(
                        c + 1, holder, ("xq", c + 1), wq_sb, bq_sb, qhT
                    )
                    if c == 0:
                        # round 0 races the prologue DMAs: xq1 cannot be on
                        # chip before the attention finishes, so its
                        # projection must not sit in the PE queue mid-round
                        post += qp + [t_wo]
                    else:
                        filler += qp
                    # K projection of chunk c+1 runs inside round c+1 itself
                    # (khT isn't needed until its diagonal), keeping PE filler
                    # in the late, exp-heavy rounds
                    kv_deferred[c + 1] = proj_qk_thunks(
                        c + 1, holder, ("xk", c + 1), wk_sb, bk_sb, khT
                    )
                # V projection of chunk c zips into the diagonal section
                vmb = proj_v_thunks(c, holder, ("xv", c))
                att = attn_thunks(c, vmb_thunks=vmb)
                # seg1 ends where hp0's diagonal section begins (the first
                # zipped v thunk); kv_filler must complete within seg1
                nsc_off = 2 * (4 * c) - (1 if c > 0 else 0)
                seg1, seg2 = att[:nsc_off] if c > 0 else [], att[nsc_off:] if c > 0 else att
                emit_interleaved(seg1, kv_filler)
                # hold back a quarter of the filler to keep PE fed through the
                # end-of-round normalization chains
                cut = (3 * len(filler)) // 4
                emit_interleaved(seg2[:-8], filler[:cut])
                emit_interleaved(seg2[-8:], filler[cut:])
                for t in post:
                    t()
            for t in wo_tail_thunks(QC - 1):
                t()

    nc.compile()
    return nc


def _get_nc(nc_s=S):
    if nc_s not in _CACHE:
        _CACHE[nc_s] = _build(nc_s)
    return _CACHE[nc_s]


def make_masks():
    # one 128x128 lower-triangle wedge (same for every diagonal block),
    # duplicated for the two heads an exp tile carries
    kl = np.arange(P)[:, None]
    t = np.arange(P)[None, :]
    m = (t >= kl).astype(np.float32)
    return np.concatenate([m, m], axis=1)


def make_in_maps(q, k, v, Wq, bq, Wk, bk, Wv, Wo):
    masks = make_masks()
    in_maps = []
    for core in range(8):
        b, g = divmod(core, 4)
        cs = slice(g * CW, (g + 1) * CW)
        in_maps.append(
            {
                "xqt": np.ascontiguousarray(q[b].T).astype(np.float16),
                "xkt": np.ascontiguousarray(k[b].T).astype(np.float16),
                "xvt": np.ascontiguousarray(v[b].T).astype(np.float16),
                "wq": np.ascontiguousarray(Wq[:, cs]).astype(np.float16),
                "wk": np.ascontiguousarray(Wk[:, cs]).astype(np.float16),
                "wv": np.ascontiguousarray(Wv[:, cs]).astype(np.float16),
                "wo": np.ascontiguousarray(Wo[cs, :]).astype(np.float16),
                "bq": np.ascontiguousarray(bq[cs]),
                "bk": np.ascontiguousarray(bk[cs]),
                "masks": masks.astype(np.float16),
                "ones": np.ones((P, P), np.float16),
            }
        )
    return in_maps


def run(q, k, v, Wq, bq, Wk, bk, Wv, bv, Wo, bo, **run_kwargs):
    """Returns (output, BassKernelResults)."""
    from concourse.bass_utils import run_bass_kernel_spmd

    q, k, v = (np.asarray(x, np.float32) for x in (q, k, v))
    Wo = np.asarray(Wo, np.float32)
    bv = np.asarray(bv, np.float32)
    nc = _get_nc()
    in_maps = make_in_maps(
        q, k, v,
        np.asarray(Wq, np.float32), np.asarray(bq, np.float32),
        np.asarray(Wk, np.float32), np.asarray(bk, np.float32),
        np.asarray(Wv, np.float32),
        Wo,
    )
    res = run_bass_kernel_spmd(nc, in_maps, list(range(8)), **run_kwargs)
    out = np.zeros((B, S, E), np.float32)
    for core in range(8):
        out[core // 4] += res.results[core]["out"].astype(np.float32)
    # V bias commutes through the softmax average; Wo is linear in it.
    out += (np.asarray(bo, np.float32) + bv @ Wo)[None, None, :]
    return out, res


def kernel(q, k, v, Wq, bq, Wk, bk, Wv, bv, Wo, bo):
    return run(q, k, v, Wq, bq, Wk, bk, Wv, bv, Wo, bo)[0]
